# revision 1
# baseline (speedup 1.0000x reference)
"""Trainium2 Bass kernel for the CriticalField PDE step.

Computes one explicit step of a coupled magnitude/phase field update on a
4096x4096 grid with circular boundary conditions:

    mag_lap   = 4-neighbor circular Laplacian of magnitude
    phase_lap = 4-neighbor circular Laplacian of phase
    d_mag     = tension*mag_lap - damping*mag - nonlinearity*mag^3
    d_phase   = tension*phase_lap + COUPLING*sin(up(phase) - phase)
    out[0]    = clip(mag + DT*d_mag, -2, 2)
    out[1]    = clip(phase + DT*d_phase, 0, 2*pi)

Sharding: rows are split across 8 NeuronCores. Each core processes 504 rows
as 4 tiles of 128 partitions (126 valid output rows each, tiles advance by
126 so the +-1 row stencil reach stays inside the tile), plus 1/8 of the 64
leftover rows (4032..4095) as a column-split "overflow" block. All halos
(row and column, circular) are materialized host-side so the device kernel
needs no collectives and no wrap logic.

Per-core compute strategy (memory-bound target):
  - TensorE: raw 4-neighbor sums + the phase roll-difference via float32r
    matmuls with {0,+-1} banded matrices accumulated in PSUM (row-direction
    neighbors via off-diagonal bands over partitions, column-direction
    neighbors via column-shifted rhs views).
  - ScalarE: square(mag), A2*phase, sin(psum_arg).
  - GpSimd:  -C*mag^3 chain step and the two output clips.
  - VectorE: the three fused scalar_tensor_tensor merges + one.
All scale factors (A, B, ...) are applied as exact fp32 immediates outside
the PE so float32r only ever multiplies data by exactly-representable 1.0.
"""

import numpy as np

SIZE = 4096
NCORES = 8
TILE_VALID = 126
NTILES = 4
MAIN_ROWS = TILE_VALID * NTILES          # 504 rows per core via main tiles
OVF_ROWS = SIZE - MAIN_ROWS * NCORES     # 64 leftover rows (4032..4095)
OVF_COLS = SIZE // NCORES                # 512 columns of overflow per core
DT = 0.05
COUPLING = 0.015
TWO_PI = 2.0 * np.pi

_PROG_CACHE: dict = {}
_WEIGHTS_CACHE: dict = {}


def _banded_weights(tension):
    """lhsT weight matrices for nc.tensor.matmul (out = lhsT.T @ rhs).

    lhsT[k, m] = contribution of rhs partition k to output partition m.
    Output partition m corresponds to slab row t+m; its row-neighbors are
    tile partitions m-1 (up) and m+1 (down). Block 3 is (COUPLING/tension)*I,
    used to inject sin(arg) into the phase PSUM so the whole PSUM can be
    scaled by B = DT*tension in one fused merge.
    """
    key = float(tension)
    if key in _WEIGHTS_CACHE:
        return _WEIGHTS_CACHE[key]
    w_ud = np.zeros((128, 128), np.float32)
    idx = np.arange(127)
    w_ud[idx, idx + 1] = 1.0      # k = m-1 -> up neighbor
    w_ud[idx + 1, idx] = 1.0      # k = m+1 -> down neighbor
    w_eye = np.eye(128, dtype=np.float32)
    w_umi = np.zeros((128, 128), np.float32)
    w_umi[idx, idx + 1] = 1.0     # +up
    w_umi[np.arange(128), np.arange(128)] = -1.0  # -center
    w_sin = np.eye(128, dtype=np.float32) * (COUPLING / tension)
    _WEIGHTS_CACHE[key] = {
        "w_all": np.ascontiguousarray(
            np.concatenate([w_ud, w_eye, w_umi, w_sin], axis=1))}
    return _WEIGHTS_CACHE[key]


def _build_program(A, B, Cc, A2, K, repeat=1, mode="full"):
    import concourse.bass as bass
    import concourse.bacc as bacc
    import concourse.tile as tile
    from concourse import mybir

    f32 = mybir.dt.float32
    f32r = mybir.dt.float32r
    Act = mybir.ActivationFunctionType
    Alu = mybir.AluOpType

    nc = bacc.Bacc(trn_type="TRN2", target_bir_lowering=False, debug=False)

    # Field slabs are declared float32r (same bits as f32) so the PE may
    # consume them directly; non-matmul consumers bitcast back to f32.
    mag_slab = nc.dram_tensor("mag_slab", [MAIN_ROWS + 2, SIZE + 2], f32r,
                              kind="ExternalInput").ap()
    ph_slab = nc.dram_tensor("ph_slab", [MAIN_ROWS + 2, SIZE + 2], f32r,
                             kind="ExternalInput").ap()
    mag_ovf = nc.dram_tensor("mag_ovf", [OVF_ROWS + 2, OVF_COLS + 2], f32r,
                             kind="ExternalInput").ap()
    ph_ovf = nc.dram_tensor("ph_ovf", [OVF_ROWS + 2, OVF_COLS + 2], f32r,
                            kind="ExternalInput").ap()
    w_all_d = nc.dram_tensor("w_all", [128, 512], f32r, kind="ExternalInput").ap()
    out_main = nc.dram_tensor("out_main", [2, MAIN_ROWS, SIZE], f32,
                              kind="ExternalOutput").ap()
    out_ovf = nc.dram_tensor("out_ovf", [2, OVF_ROWS, OVF_COLS], f32,
                             kind="ExternalOutput").ap()

    with tile.TileContext(nc) as tc:
        with (
            tc.tile_pool(name="wts", bufs=1) as wpool,
            tc.tile_pool(name="inp", bufs=2) as inp,
            tc.tile_pool(name="outp", bufs=2) as outp,
            tc.tile_pool(name="tmp", bufs=2) as tmp,
            tc.tile_pool(name="sml", bufs=4) as sml,
            tc.tile_pool(name="psm", bufs=3, space="PSUM") as psm,
            tc.tile_pool(name="psb", bufs=2, space="PSUM") as psb,
        ):
            w_all = wpool.tile([128, 512], f32r, tag="w_all")
            nc.sync.dma_start(w_all[:, :], w_all_d[:, :])

            def emit_block(mg, ph, om, op_, P, ncols):
                """Emit compute for one loaded tile.

                mg/ph: input tiles [P, ncols+2] (col halo included)
                om/op_: output tiles [P, ncols]; valid partitions 1..P-2.
                mode ladder (timing diagnostics): "dma" = loads/stores only;
                "pe" = +matmuls; "peact" = +ScalarE ops; "full" = everything.
                """
                if mode == "dma":
                    nc.vector.tensor_copy(om[0:P, 0:ncols],
                                          mg[0:P, 1:1 + ncols].bitcast(f32))
                    nc.gpsimd.tensor_copy(op_[0:P, 0:ncols],
                                          ph[0:P, 1:1 + ncols].bitcast(f32))
                    return
                do_act = mode in ("peact", "full")
                do_rest = mode == "full"
                wud = w_all[0:P, 0:P]
                weye = w_all[0:P, 128:128 + P]
                wumi = w_all[0:P, 256:256 + P]
                wsin = w_all[0:P, 384:384 + P]
                nblk = (ncols + 1023) // 1024
                for b in range(nblk):
                    b0 = 1024 * b
                    bw = min(1024, ncols - b0)
                    magc = mg[0:P, 1 + b0:1 + b0 + bw].bitcast(f32)
                    phc = ph[0:P, 1 + b0:1 + b0 + bw].bitcast(f32)
                    if do_act:
                        c2 = tmp.tile([P, bw], f32, tag="c2")
                        nc.scalar.activation(c2[:, :], magc, Act.Square,
                                             bias=0.0, scale=float(np.sqrt(Cc)))
                        t2 = tmp.tile([P, bw], f32, tag="t2")
                        nc.scalar.activation(t2[:, :], phc, Act.Copy,
                                             bias=0.0, scale=A2)
                    if do_rest:
                        c3t = tmp.tile([P, bw], f32, tag="c3t")
                        nc.gpsimd.tensor_tensor(
                            c3t[:, :], c2[:, :], magc, Alu.mult)
                        tmg = tmp.tile([P, bw], f32, tag="tmg")
                        nc.vector.scalar_tensor_tensor(
                            tmg[:, :], magc, A, c3t[:, :], Alu.mult, Alu.subtract)

                    for j in range(0, bw, 512):
                        c0 = b0 + j
                        cw = min(512, bw - j)
                        mg_c = mg[0:P, 1 + c0:1 + c0 + cw]
                        mg_l = mg[0:P, c0:c0 + cw]
                        mg_r = mg[0:P, 2 + c0:2 + c0 + cw]
                        ph_c = ph[0:P, 1 + c0:1 + c0 + cw]
                        ph_l = ph[0:P, c0:c0 + cw]
                        ph_r = ph[0:P, 2 + c0:2 + c0 + cw]

                        pm = psm.tile([P, cw], f32, tag="pm")
                        nc.tensor.matmul(pm[:, :], wud, mg_c, start=True, stop=False)
                        nc.tensor.matmul(pm[:, :], weye, mg_l, start=False, stop=False)
                        nc.tensor.matmul(pm[:, :], weye, mg_r, start=False, stop=True)
                        pa = psb.tile([P, cw], f32, tag="pa")
                        nc.tensor.matmul(pa[:, :], wumi, ph_c, start=True, stop=True)
                        pp = psm.tile([P, cw], f32, tag="pp")
                        nc.tensor.matmul(pp[:, :], wud, ph_c, start=True, stop=False)
                        nc.tensor.matmul(pp[:, :], weye, ph_l, start=False, stop=False)
                        if not do_act:
                            nc.tensor.matmul(pp[:, :], weye, ph_r,
                                             start=False, stop=True)
                            continue
                        nc.tensor.matmul(pp[:, :], weye, ph_r,
                                         start=False, stop=False)
                        s = sml.tile([P, cw], f32r, tag="s")
                        nc.scalar.activation(s[:, :], pa[:, :], Act.Sin)
                        nc.tensor.matmul(pp[:, :], wsin, s[:, :],
                                         start=False, stop=True)
                        if not do_rest:
                            continue
                        mm = sml.tile([P, cw], f32, tag="mm")
                        nc.vector.scalar_tensor_tensor(
                            mm[:, :], pm[:, :], B, tmg[:, j:j + cw],
                            Alu.mult, Alu.add)
                        m2a = sml.tile([P, cw], f32, tag="m2a")
                        nc.vector.scalar_tensor_tensor(
                            m2a[:, :], pp[:, :], B, t2[:, j:j + cw],
                            Alu.mult, Alu.add)
                        nc.vector.tensor_scalar(
                            om[0:P, c0:c0 + cw], mm[0:P, :],
                            2.0, -2.0, Alu.min, Alu.max)
                        nc.gpsimd.tensor_scalar(
                            op_[0:P, c0:c0 + cw], m2a[0:P, :],
                            0.0, float(np.float32(TWO_PI)), Alu.max, Alu.min)
                if mode in ("pe", "peact"):
                    nc.vector.tensor_copy(om[0:P, 0:ncols],
                                          mg[0:P, 1:1 + ncols].bitcast(f32))
                    nc.gpsimd.tensor_copy(op_[0:P, 0:ncols],
                                          ph[0:P, 1:1 + ncols].bitcast(f32))

            HALF = SIZE // 2
            for _rep in range(repeat):
              # Overflow block first: its small ops fill the pipeline-fill
              # bubble while the first big tile's DMA is still in flight.
              P = OVF_ROWS + 2
              mg = inp.tile([P, OVF_COLS + 2], f32r, tag="mg")
              nc.sync.dma_start(mg[:, :], mag_ovf[:, :])
              ph = inp.tile([P, OVF_COLS + 2], f32r, tag="ph")
              nc.sync.dma_start(ph[:, :], ph_ovf[:, :])
              om = outp.tile([P, OVF_COLS], f32, tag="om")
              op_ = outp.tile([P, OVF_COLS], f32, tag="op")
              emit_block(mg, ph, om, op_, P, OVF_COLS)
              nc.sync.dma_start(out_ovf[0, :, :], om[1:P - 1, :])
              nc.sync.dma_start(out_ovf[1, :, :], op_[1:P - 1, :])

              for ti in range(NTILES):
                t0 = TILE_VALID * ti
                mg = inp.tile([128, SIZE + 2], f32r, tag="mg")
                nc.sync.dma_start(mg[:, :], mag_slab[t0:t0 + 128, :])
                ph = inp.tile([128, SIZE + 2], f32r, tag="ph")
                nc.sync.dma_start(ph[:, :], ph_slab[t0:t0 + 128, :])
                om = outp.tile([128, SIZE], f32, tag="om")
                op_ = outp.tile([128, SIZE], f32, tag="op")
                emit_block(mg, ph, om, op_, 128, SIZE)
                # Drain each output in column halves so the store of the
                # first half overlaps the clips of the second.
                for lo in (0, HALF):
                    nc.sync.dma_start(
                        out_main[0, t0:t0 + TILE_VALID, lo:lo + HALF],
                        om[1:127, lo:lo + HALF])
                    nc.sync.dma_start(
                        out_main[1, t0:t0 + TILE_VALID, lo:lo + HALF],
                        op_[1:127, lo:lo + HALF])

    nc.compile()
    return nc


def _get_program(damping, tension, nonlinearity, repeat=1, mode="full"):
    key = (damping, tension, nonlinearity, repeat, mode)
    if key not in _PROG_CACHE:
        A = 1.0 - 4.0 * DT * tension - DT * damping
        B = DT * tension
        Cc = DT * nonlinearity
        A2 = 1.0 - 4.0 * DT * tension
        K = DT * COUPLING
        _PROG_CACHE[key] = _build_program(A, B, Cc, A2, K, repeat, mode)
    return _PROG_CACHE[key]


def _make_in_maps(mag, ph, tension=1.5):
    """Build per-core input dicts with all circular halos materialized."""
    w = _banded_weights(tension)
    cols = np.arange(-1, SIZE + 1) % SIZE
    ovf_rows = np.arange(MAIN_ROWS * NCORES - 1, SIZE + 1) % SIZE
    mag_ovf_full = mag[np.ix_(ovf_rows, cols)]
    ph_ovf_full = ph[np.ix_(ovf_rows, cols)]
    in_maps = []
    for m in range(NCORES):
        rows = np.arange(MAIN_ROWS * m - 1, MAIN_ROWS * (m + 1) + 1) % SIZE
        c0 = OVF_COLS * m
        in_maps.append({
            "mag_slab": np.ascontiguousarray(mag[np.ix_(rows, cols)]),
            "ph_slab": np.ascontiguousarray(ph[np.ix_(rows, cols)]),
            "mag_ovf": np.ascontiguousarray(mag_ovf_full[:, c0:c0 + OVF_COLS + 2]),
            "ph_ovf": np.ascontiguousarray(ph_ovf_full[:, c0:c0 + OVF_COLS + 2]),
            "w_all": w["w_all"],
        })
    return in_maps


def _assemble(results):
    out = np.empty((1, 2, SIZE, SIZE), np.float32)
    for m in range(NCORES):
        r = results[m]
        out[0, :, MAIN_ROWS * m:MAIN_ROWS * (m + 1), :] = r["out_main"]
        out[0, :, MAIN_ROWS * NCORES:, OVF_COLS * m:OVF_COLS * (m + 1)] = \
            r["out_ovf"]
    return out


def kernel(magnitude, phase, damping, tension, nonlinearity):
    from concourse.bass_utils import run_bass_kernel_spmd

    mag = np.asarray(magnitude, dtype=np.float32).reshape(SIZE, SIZE)
    ph = np.asarray(phase, dtype=np.float32).reshape(SIZE, SIZE)
    d = float(np.asarray(damping))
    tn = float(np.asarray(tension))
    nl = float(np.asarray(nonlinearity))

    nc = _get_program(d, tn, nl)
    in_maps = _make_in_maps(mag, ph, tn)
    res = run_bass_kernel_spmd(nc, in_maps, core_ids=list(range(NCORES)))
    return _assemble(res.results)



# revision 26
# speedup vs baseline: 15.5448x; 15.5448x over previous
"""Trainium2 Bass kernel for the CriticalField PDE step.

Computes one explicit step of a coupled magnitude/phase field update on a
4096x4096 grid with circular boundary conditions:

    mag_lap   = 4-neighbor circular Laplacian of magnitude
    phase_lap = 4-neighbor circular Laplacian of phase
    d_mag     = tension*mag_lap - damping*mag - nonlinearity*mag^3
    d_phase   = tension*phase_lap + COUPLING*sin(up(phase) - phase)
    out[0]    = clip(mag + DT*d_mag, -2, 2)
    out[1]    = clip(phase + DT*d_phase, 0, 2*pi)

Sharding: rows split across 8 NeuronCores; each core runs 4 tiles of 128
partitions (126 valid rows, +-1 row halo inside the tile) plus 1/8 of the 64
leftover rows as a column-split overflow block. All circular halos are
materialized host-side; no collectives.

v2 (fp16): all field data moves as fp16 (loads, stores, SBUF intermediates),
halving HBM traffic vs f32 — the memory-bound roofline. mag|ph are
interleaved per row in one slab so each tile is ONE ~2.1MB load and ONE
~2.06MB store. Row-neighbor sums use PE band matmuls with the center term
folded into the band diagonal:

    pm = (B*offdiag + A*diag) @ mag  -  I @ (Cc*mag^3)        [PSUM, f32]
    pp = (B*offdiag + A2*diag) @ ph  +  K*I @ sin(up-ph)      [PSUM, f32]
    pa = (up - center) @ ph                                    [sin argument]

so the only non-PE work per field is one column-neighbor add (l+r views),
one scalar_tensor_tensor merge (B*lr + psum) and one clip, balanced across
DVE/GpSimd; ScalarE does Square (for the cubic) and Sin.
"""

import numpy as np

SIZE = 4096
NCORES = 8
TILE_VALID = 126
NTILES = 4
MAIN_ROWS = TILE_VALID * NTILES          # 504 rows per core via main tiles
OVF_ROWS = SIZE - MAIN_ROWS * NCORES     # 64 leftover rows (4032..4095)
OVF_COLS = SIZE // NCORES                # 512 columns of overflow per core
DT = 0.05
COUPLING = 0.015
TWO_PI = 2.0 * np.pi

_PROG_CACHE: dict = {}
_WEIGHTS_CACHE: dict = {}


def _banded_weights(damping, tension):
    """fp16 lhsT weight matrices for nc.tensor.matmul (out = lhsT.T @ rhs).

    lhsT[k, m] = contribution of rhs partition k to output partition m.
    Four 128x128 blocks, packed side by side:
      0: wudA  = B on both off-diagonals + A on the diagonal   (mag row pass)
      1: wudA2 = B on both off-diagonals + A2 on the diagonal  (phase row pass)
      2: wumi  = +1 at k=m-1, -1 on diagonal                   (sin argument)
      3: wneg | wK = -1 diagonal (cols 0..) and K diagonal     (injections)
    wneg/wK share block 3: wneg is -I, wK is K*I; they are built as two
    separate quarters 3 and 4.
    """
    key = (float(damping), float(tension))
    if key in _WEIGHTS_CACHE:
        return _WEIGHTS_CACHE[key]
    A = 1.0 - 4.0 * DT * tension - DT * damping
    A2 = 1.0 - 4.0 * DT * tension
    B = DT * tension
    K = DT * COUPLING
    idx = np.arange(127)
    w_udA = np.zeros((128, 128), np.float32)
    w_udA[idx, idx + 1] = B
    w_udA[idx + 1, idx] = B
    w_udA[np.arange(128), np.arange(128)] = A
    w_udA2 = np.zeros((128, 128), np.float32)
    w_udA2[idx, idx + 1] = B
    w_udA2[idx + 1, idx] = B
    w_udA2[np.arange(128), np.arange(128)] = A2
    w_umi = np.zeros((128, 128), np.float32)
    w_umi[idx, idx + 1] = 1.0
    w_umi[np.arange(128), np.arange(128)] = -1.0
    w_neg = -B * np.eye(128, dtype=np.float32)
    w_K = np.eye(128, dtype=np.float32) * K
    w_B = np.eye(128, dtype=np.float32) * B
    w_all = np.concatenate([w_udA, w_udA2, w_umi, w_neg, w_K, w_B], axis=1)
    _WEIGHTS_CACHE[key] = np.ascontiguousarray(w_all.astype(np.float16))
    return _WEIGHTS_CACHE[key]


def _build_program(A, B, Cc, A2, K, repeat=1, mode="full"):
    import concourse.bass as bass
    import concourse.bacc as bacc
    import concourse.tile as tile
    from concourse import mybir

    f32 = mybir.dt.float32
    f16 = mybir.dt.float16
    Act = mybir.ActivationFunctionType
    Alu = mybir.AluOpType

    nc = bacc.Bacc(trn_type="TRN2", target_bir_lowering=False, debug=False)

    W2 = 2 * SIZE + 4                     # interleaved slab width (mag|ph)
    WO2 = 2 * OVF_COLS + 4
    slab = nc.dram_tensor("slab", [MAIN_ROWS + 2, W2], f16,
                          kind="ExternalInput").ap()
    ovf = nc.dram_tensor("ovf", [OVF_ROWS + 2, WO2], f16,
                         kind="ExternalInput").ap()
    w_all_d = nc.dram_tensor("w_all", [128, 768], f16, kind="ExternalInput").ap()
    out_main = nc.dram_tensor("out_main", [MAIN_ROWS, 2 * SIZE], f16,
                              kind="ExternalOutput").ap()
    out_ovf = nc.dram_tensor("out_ovf", [OVF_ROWS, 2 * OVF_COLS], f16,
                             kind="ExternalOutput").ap()

    sqrtCc = float(np.sqrt(Cc / B))   # c2 = (Cc/B)*mag^2; all paths reinject *B
    two_pi16 = float(np.float16(TWO_PI))

    with tile.TileContext(nc) as tc:
        with (
            tc.tile_pool(name="wts", bufs=1) as wpool,
            tc.tile_pool(name="inp", bufs=4) as inp,
            tc.tile_pool(name="outp", bufs=2) as outp,
            tc.tile_pool(name="tmp", bufs=3) as tmp,
            tc.tile_pool(name="sml", bufs=3) as sml,
            tc.tile_pool(name="psm", bufs=3, space="PSUM") as psm,
            tc.tile_pool(name="psb", bufs=3, space="PSUM") as psb,
            tc.tile_pool(name="psa", bufs=2, space="PSUM") as psa,
        ):
            w_all = wpool.tile([128, 768], f16, tag="w_all")
            nc.sync.dma_start(w_all[:, :], w_all_d[:, :])

            def emit_block(IT, OT, P, ncols):
                """Emit compute for one loaded interleaved tile.

                IT: input tile [P, 2*ncols+4]; mag cols 0:ncols+2,
                    ph cols ncols+2 : 2*ncols+4 (each with 1-col halo).
                OT: output tile [P, 2*ncols]; valid partitions 1..P-2.
                mode ladder: "dma" = loads/stores only; "pe" = +matmuls;
                "peact" = +ScalarE; "full" = everything.
                """
                mg = IT[0:P, 0:ncols + 2]
                ph = IT[0:P, ncols + 2:2 * ncols + 4]
                if mode == "dma":
                    nc.vector.tensor_copy(OT[0:P, 0:ncols], mg[0:P, 1:1 + ncols])
                    nc.gpsimd.tensor_copy(OT[0:P, ncols:2 * ncols],
                                          ph[0:P, 1:1 + ncols])
                    return
                do_act = mode in ("peact", "full")
                do_rest = mode == "full"
                wudA = w_all[0:P, 0:P]
                wudA2 = w_all[0:P, 128:128 + P]
                wumi = w_all[0:P, 256:256 + P]
                wneg = w_all[0:P, 384:384 + P]
                wK = w_all[0:P, 512:512 + P]
                wB = w_all[0:P, 640:640 + P]

                for b0 in range(0, ncols, 1024):
                    bw = min(1024, ncols - b0)
                    if do_rest:
                        # SBUF-only prep: column-neighbor sums on GpSimd (no
                        # PSUM port there), cubic term via ScalarE square +
                        # GpSimd multiply. Emit lr before c3t so the GpSimd
                        # FIFO head never blocks on ScalarE.
                        lr_m = tmp.tile([P, bw], f16, tag="lrm")
                        nc.gpsimd.tensor_tensor(lr_m[:, :], mg[0:P, b0:b0 + bw],
                                                mg[0:P, b0 + 2:b0 + bw + 2],
                                                Alu.add)
                        lr_p = tmp.tile([P, bw], f16, tag="lrp")
                        nc.gpsimd.tensor_tensor(lr_p[:, :], ph[0:P, b0:b0 + bw],
                                                ph[0:P, b0 + 2:b0 + bw + 2],
                                                Alu.add)
                        c2 = tmp.tile([P, bw], f16, tag="c2")
                        nc.scalar.activation(c2[:, :], mg[0:P, 1 + b0:1 + b0 + bw],
                                             Act.Square, bias=0.0, scale=sqrtCc)
                        c3t = tmp.tile([P, bw], f16, tag="c3t")
                        nc.gpsimd.tensor_tensor(c3t[:, :], c2[:, :],
                                                mg[0:P, 1 + b0:1 + b0 + bw],
                                                Alu.mult)
                    fold = False

                    # Per-512-chunk PSUM groups (bufs=3 each for loose
                    # pipelining). pm collects the COMPLETE mag update, pp the
                    # complete phase update, so the only post-PSUM op is the
                    # clip (DVE, straight PSUM -> OT). IT-only matmuls are
                    # emitted before the wneg/wK ones that depend on
                    # GpSimd/ScalarE products.
                    nj = range(0, bw, 512)
                    pms, pps = {}, {}
                    for j in nj:
                        cw = min(512, bw - j)
                        ph_c = ph[0:P, 1 + b0 + j:1 + b0 + j + cw]
                        pa = psa.tile([P, cw], f32, tag="pa")
                        nc.tensor.matmul(pa[:, :], wumi, ph_c,
                                         start=True, stop=True)
                        if do_act:
                            s = sml.tile([P, cw], f16, tag="s")
                            nc.scalar.activation(s[:, :], pa[:, :], Act.Sin)
                        else:
                            s = None
                        pms[j] = psm.tile([P, cw], f32, tag="pm", name="pm")
                        pps[j] = (psb.tile([P, cw], f32, tag="pp", name="pp"), s)
                    for j in nj:
                        cw = min(512, bw - j)
                        mg_c = mg[0:P, 1 + b0 + j:1 + b0 + j + cw]
                        last = not do_rest
                        nc.tensor.matmul(pms[j][:, :], wudA, mg_c,
                                         start=True, stop=last)
                        if do_rest:
                            nc.tensor.matmul(pms[j][:, :], wB,
                                             lr_m[:, j:j + cw],
                                             start=False, stop=fold)
                            if not fold:
                                nc.tensor.matmul(pms[j][:, :], wneg,
                                                 c3t[:, j:j + cw],
                                                 start=False, stop=True)
                    for j in nj:
                        cw = min(512, bw - j)
                        ph_c = ph[0:P, 1 + b0 + j:1 + b0 + j + cw]
                        pp, s = pps[j]
                        last = not do_rest
                        nc.tensor.matmul(pp[:, :], wudA2, ph_c,
                                         start=True, stop=last)
                        if do_rest:
                            nc.tensor.matmul(pp[:, :], wB, lr_p[:, j:j + cw],
                                             start=False, stop=False)
                            nc.tensor.matmul(pp[:, :], wK, s[:, :],
                                             start=False, stop=True)
                    for j in nj:
                        cw = min(512, bw - j)
                        c0 = b0 + j
                        if do_rest:
                            nc.vector.tensor_scalar(
                                OT[0:P, c0:c0 + cw], pms[j][:, :],
                                2.0, -2.0, Alu.min, Alu.max)
                        else:
                            nc.vector.tensor_copy(OT[0:P, c0:c0 + cw],
                                                  pms[j][:, :])
                            nc.gpsimd.tensor_copy(
                                OT[0:P, ncols + c0:ncols + c0 + cw],
                                mg[0:P, 1 + c0:1 + c0 + cw])
                    if do_rest:
                        for j in nj:
                            cw = min(512, bw - j)
                            c0 = b0 + j
                            nc.vector.tensor_scalar(
                                OT[0:P, ncols + c0:ncols + c0 + cw],
                                pps[j][0][:, :],
                                0.0, two_pi16, Alu.max, Alu.min)

            for _rep in range(repeat):
                # Overflow block first: its small ops fill the pipeline-fill
                # bubble while the first big tile's DMA is still in flight.
                P = OVF_ROWS + 2
                ITo = inp.tile([P, WO2], f16, tag="ito")
                nc.sync.dma_start(ITo[:, :], ovf[:, :])
                # All main-tile loads issued up front (SP executes its queue
                # in order; loads must not sit behind stores). Each tile loads
                # as 4 column strips (2 per field, 2-col overlap) so block 0's
                # compute can start after ~1/4 of the tile has landed.
                ITs = []
                for ti in range(NTILES):
                    t0 = TILE_VALID * ti
                    IT = inp.tile([128, W2], f16, tag="it", name="it")
                    # Phase half first: the pa -> sin -> wK chain is the
                    # longest dependency path, so its data should land first.
                    for f0 in (SIZE + 2, 0):
                        half = (SIZE + 2) // 2 + 1      # 2050
                        nc.sync.dma_start(IT[:, f0:f0 + half],
                                          slab[t0:t0 + 128, f0:f0 + half])
                        nc.sync.dma_start(
                            IT[:, f0 + half - 2:f0 + SIZE + 2],
                            slab[t0:t0 + 128, f0 + half - 2:f0 + SIZE + 2])
                    ITs.append(IT)

                OTo = outp.tile([P, 2 * OVF_COLS], f16, tag="oto")
                emit_block(ITo, OTo, P, OVF_COLS)
                nc.sync.dma_start(out_ovf[:, :], OTo[1:P - 1, :])

                for ti in range(NTILES):
                    t0 = TILE_VALID * ti
                    OT = outp.tile([128, 2 * SIZE], f16, tag="ot")
                    emit_block(ITs[ti], OT, 128, SIZE)
                    # Store in 2048-col strips, emitted in completion order
                    # (mag blocks 0-1, ph blocks 0-1, mag 2-3, ph 2-3) so the
                    # SP FIFO never holds a ready strip behind an unready one.
                    for s0 in (0, SIZE, 2048, SIZE + 2048):
                        nc.sync.dma_start(
                            out_main[t0:t0 + TILE_VALID, s0:s0 + 2048],
                            OT[1:127, s0:s0 + 2048])

    nc.compile()
    return nc


def _get_program(damping, tension, nonlinearity, repeat=1, mode="full"):
    key = (damping, tension, nonlinearity, repeat, mode)
    if key not in _PROG_CACHE:
        A = 1.0 - 4.0 * DT * tension - DT * damping
        B = DT * tension
        Cc = DT * nonlinearity
        A2 = 1.0 - 4.0 * DT * tension
        K = DT * COUPLING
        _PROG_CACHE[key] = _build_program(A, B, Cc, A2, K, repeat, mode)
    return _PROG_CACHE[key]


def _make_in_maps(mag, ph, damping=0.05, tension=1.5):
    """Per-core input dicts: fp16 interleaved slabs with circular halos."""
    w_all = _banded_weights(damping, tension)
    m16 = mag.astype(np.float16)
    p16 = ph.astype(np.float16)
    # Padded interleaved array: rows -1..4096, cols [mag -1..4096 | ph -1..4096]
    W2 = 2 * SIZE + 4
    P = np.empty((SIZE + 2, W2), np.float16)
    for base, f in ((0, m16), (SIZE + 2, p16)):
        P[1:-1, base + 1:base + 1 + SIZE] = f
        P[0, base + 1:base + 1 + SIZE] = f[-1]
        P[-1, base + 1:base + 1 + SIZE] = f[0]
        P[:, base] = P[:, base + SIZE]          # left halo = col 4095
        P[:, base + SIZE + 1] = P[:, base + 1]  # right halo = col 0
    in_maps = []
    for m in range(NCORES):
        r0 = MAIN_ROWS * m
        c0 = OVF_COLS * m
        ovf = np.concatenate(
            [P[MAIN_ROWS * NCORES:SIZE + 2, c0:c0 + OVF_COLS + 2],
             P[MAIN_ROWS * NCORES:SIZE + 2,
               SIZE + 2 + c0:SIZE + 2 + c0 + OVF_COLS + 2]], axis=1)
        in_maps.append({
            "slab": np.ascontiguousarray(P[r0:r0 + MAIN_ROWS + 2, :]),
            "ovf": np.ascontiguousarray(ovf),
            "w_all": w_all,
        })
    return in_maps


def _assemble(results):
    out = np.empty((1, 2, SIZE, SIZE), np.float32)
    for m in range(NCORES):
        r = results[m]
        main = r["out_main"].astype(np.float32)
        out[0, 0, MAIN_ROWS * m:MAIN_ROWS * (m + 1), :] = main[:, :SIZE]
        out[0, 1, MAIN_ROWS * m:MAIN_ROWS * (m + 1), :] = main[:, SIZE:]
        o = r["out_ovf"].astype(np.float32)
        out[0, 0, MAIN_ROWS * NCORES:, OVF_COLS * m:OVF_COLS * (m + 1)] = \
            o[:, :OVF_COLS]
        out[0, 1, MAIN_ROWS * NCORES:, OVF_COLS * m:OVF_COLS * (m + 1)] = \
            o[:, OVF_COLS:]
    return out


def kernel(magnitude, phase, damping, tension, nonlinearity):
    from concourse.bass_utils import run_bass_kernel_spmd

    mag = np.asarray(magnitude, dtype=np.float32).reshape(SIZE, SIZE)
    ph = np.asarray(phase, dtype=np.float32).reshape(SIZE, SIZE)
    d = float(np.asarray(damping))
    tn = float(np.asarray(tension))
    nl = float(np.asarray(nonlinearity))

    nc = _get_program(d, tn, nl)
    in_maps = _make_in_maps(mag, ph, d, tn)
    res = run_bass_kernel_spmd(nc, in_maps, core_ids=list(range(NCORES)))
    return _assemble(res.results)
